# revision 44
# baseline (speedup 1.0000x reference)
"""GQA attention layer (B=2, S=2048, D=4096, 32 Q heads / 8 KV heads, HD=128)
with rotary embeddings, causal mask, and output projection, on 8 trn2 cores.

Sharding: tensor-parallel over heads for QKV+attention (core c owns Q heads
[4c,4c+4) and KV head c), two AllToAlls (split by head-pair, overlapped with
compute) to re-shard the attention output from head-sharded to token-sharded,
then token-sharded output projection with the full wo (no reduction needed).
Host gathers the 8 token shards.

The two big GEMMs (QKV projection, output projection) run as scaled 3-term
fp8e4 DoubleRow matmuls: A@B ~ [Ah@Bh + (Al@Bh + Ah@Bl)/32]/Sa/Sb with
Ah=e4(A*Sa), Al=e4((A*Sa-Ah)*32); hi and lo chains accumulate in separate
PSUMs (shared lo scale) and combine via ACT scale + DVE add (a DVE op may
read at most one PSUM operand, and a psum start=True marks the whole 2KB
bank pending-zero, so each psum tile gets exactly one chain-starting
matmul).  Inputs (x, wqkv, wo) are host-re-blocked so every DMA lands as
contiguous >=512B runs per partition (sub-512B descriptors cost 2x on the
DMA bus); loads, scratch writes and prefetches are spread over the SP/ACT/
Pool DMA queues so a waiting DMA never head-blocks an urgent one.
Attention stays bf16 (fp8 noise in q/k/v passes through at full relative
strength in this diffuse-softmax regime).  Softmax runs in transposed [k, q]
layout; exp on ACT in 1024-wide 2-block groups with the next q-tile's first
blocks pre-emitted across the boundary; denominators come from at.T @ ones
column matmuls (output free size 1 -> ~free on PE, vs a full row-matmul
pass); the [q,4]-shaped reciprocals are PE-transposed to a partition-0 row
and Pool-broadcast for the normalize multiply, deferred one block so the
chain never parks at the head of PE's in-order queue.  Causal diag blocks
use exact widths (bf16 matmuls have no sub-256 penalty); the mask lands as
a PE psum-accumulate of a constant -30000 strictly-lower triangle so exp
yields exact zeros with no cross-engine hop.  y ships as fp8 hi/lo over the
A2A (2B/elem on the wire).  Phase 4 splits each psum chain into even pairs
(first A2A) and odd pairs (second) so the output projection starts while
A2A #2 is still in flight.
"""
import sys

sys.path.insert(0, "/opt/trn_rl_repo")

import numpy as np
import ml_dtypes

B, S, D = 2, 2048, 4096
NH, NL, HD = 32, 8, 128
CORES = 8
QH = NH // CORES          # 4 q heads per core
TOK = B * S               # 4096
TPC = TOK // CORES        # 512 tokens per core (output sharding)
NT = 256                  # phase-1 token block width
KB_D = D // 128           # 32 contraction blocks over D
QT_W = 512                # phase-2 q tile width
N_QT = S // QT_W          # 4 q tiles per batch
N_KB = S // 128           # 16 k blocks per batch
WO_NT = 512               # phase-4 dout block width
SCALE = 1.0 / np.sqrt(np.float32(HD))
SW = 64.0                 # fp8 hi scale for ~N(0, 1/64) weights
SL = 32.0                 # fp8 lo-residual scale
NPAIR = KB_D // 2         # 16 kb pairs
NCH = 4                   # weight chunks (4 pairs each)
NROW = (QH + 2) * HD      # 768 qkv rows per core
NM = NROW // 128          # 6 m tiles (0..3 q heads, 4 kT, 5 vT)
NNT = TOK // NT           # 16 phase-1 token blocks

_CACHE = {}


def _build_nc(mode, c_sub, sim=False):
    """mode: 'causal' | 'full' | 'generic'. c_sub: global softmax shift.
    sim=True: single-core TimelineSim variant (collective replaced by DMAs)."""
    import concourse.bacc as bacc
    import concourse.mybir as mybir
    import concourse.tile as tile
    from contextlib import ExitStack

    F32 = mybir.dt.float32
    F32R = mybir.dt.float32r
    BF16 = mybir.dt.bfloat16
    F8 = mybir.dt.float8e4
    AT = mybir.ActivationFunctionType
    OP = mybir.AluOpType
    PM = mybir.MatmulPerfMode

    nc = bacc.Bacc("TRN2", target_bir_lowering=False, debug=False,
                   num_devices=1 if sim else CORES)

    # host-preblocked inputs: every DMA is contiguous per partition
    xTh_d = nc.dram_tensor("xTh", (NNT, 128, NPAIR * 2 * NT), F8,
                           kind="ExternalInput").ap()
    xTl_d = nc.dram_tensor("xTl", (NNT, 128, NPAIR * 2 * NT), F8,
                           kind="ExternalInput").ap()
    wqh_d = nc.dram_tensor("wqh", (NCH, 128, (NPAIR // NCH) * 2 * NROW), F8,
                           kind="ExternalInput").ap()
    wql_d = nc.dram_tensor("wql", (NCH, 128, (NPAIR // NCH) * 2 * NROW), F8,
                           kind="ExternalInput").ap()
    woh_d = nc.dram_tensor("woh", (D // WO_NT, 128, NPAIR * 2 * WO_NT), F8,
                           kind="ExternalInput").ap()
    wol_d = nc.dram_tensor("wol", (D // WO_NT, 128, NPAIR * 2 * WO_NT), F8,
                           kind="ExternalInput").ap()
    cosP_d = nc.dram_tensor("cosP", (HD, TOK), F32, kind="ExternalInput").ap()
    sinP_d = nc.dram_tensor("sinP", (HD, TOK), F32, kind="ExternalInput").ap()
    if mode == "generic":
        biasT_d = nc.dram_tensor("biasT", (S, S), F32, kind="ExternalInput").ap()
    out_d = nc.dram_tensor("out", (TPC, D), F32, kind="ExternalOutput").ap()

    ident_h = nc.inline_tensor(np.eye(128, dtype=np.float32), name="ident")
    pswap = np.zeros((128, 128), dtype=np.float32)
    for i in range(64):
        pswap[2 * i, 2 * i + 1] = -1.0
        pswap[2 * i + 1, 2 * i] = 1.0
    pswapT_h = nc.inline_tensor(np.ascontiguousarray(pswap.T), name="pswapT")
    # mask-add lhsT: out wedge += -30000 where col < row (strictly lower)
    # via matmul(psc, trinegT, ident) accumulation -- exp then yields exact 0
    trineg = np.triu(np.full((128, 128), -30000.0, np.float32), 1)
    trinegT_h = nc.inline_tensor(np.ascontiguousarray(trineg), name="trinegT")
    # 4.0 in the denominator weights knocks the y scale from 64x down to
    # 16x so extreme y elements (up to ~5*16=80) stay under the e4m3 max 240
    ones_col_h = nc.inline_tensor(np.full((128, 1), 4.0, np.float32),
                                  name="ones_col")

    causal = mode == "causal"

    with tile.TileContext(nc) as tc, ExitStack() as glob:
        dram = glob.enter_context(tc.tile_pool(name="dram", bufs=1, space="DRAM"))
        consts = glob.enter_context(tc.tile_pool(name="consts", bufs=1))

        # per-batch scratch so phase 2 (b=0) can start while phase 1 is
        # still writing b=1 (tile deps are whole-tensor)
        qkvT_t = [dram.tile([(NM - 1) * 128, S], BF16, name=f"qkvT{b}")
                  for b in range(B)]
        v_t = [dram.tile([S, HD], BF16, name=f"v{b}") for b in range(B)]
        # split A2A, fp8: chunk j = rows [512j,512j+512): hi y rows
        # [512j, 512j+256), lo rows [512j+256, 512j+512)
        a2a_in = [dram.tile([TOK, TPC], F8, name=f"a2a_in{hp}")
                  for hp in range(2)]
        a2a_out = [dram.tile([TOK, TPC], F8, name=f"a2a_out{hp}")
                   for hp in range(2)]


        # consts ride the idle SWDGE (Pool) queue: zero HWDGE contention
        # with the startup weight/x loads
        ident_sb = consts.tile([128, 128], F32)
        nc.gpsimd.dma_start(ident_sb[:], ident_h.ap())
        pswapT_sb = consts.tile([128, 128], F32R)
        nc.gpsimd.dma_start(pswapT_sb[:], pswapT_h.ap().bitcast(F32R))
        trif = consts.tile([128, 128], F32)
        nc.gpsimd.dma_start(trif[:], trinegT_h.ap())
        trinegT_sb = consts.tile([128, 128], BF16)
        nc.vector.tensor_copy(trinegT_sb[:], trif[:])
        identb_sb = consts.tile([128, 128], BF16)
        nc.vector.tensor_copy(identb_sb[:], ident_sb[:])
        ones_colf = consts.tile([128, 1], F32)
        nc.gpsimd.dma_start(ones_colf[:], ones_col_h.ap())
        ones_col_sb = consts.tile([128, 1], BF16)
        nc.scalar.copy(ones_col_sb[:], ones_colf[:])

        # phase-2 SBUF pools live in glob so their first loads (on the ACT
        # HWDGE queue) can land while phase 1 is still running
        p2_qk = glob.enter_context(tc.tile_pool(name="p2_qk", bufs=1))
        p2_v = glob.enter_context(tc.tile_pool(name="p2_v", bufs=1))
        p2_at = glob.enter_context(tc.tile_pool(name="p2_at", bufs=1))
        p2_ms = glob.enter_context(tc.tile_pool(name="p2_ms", bufs=1))

        # ========= phase 1: qkv projection (fp8 3-term) + rope + v transpose
        with ExitStack() as ctx1:
            p1_w = ctx1.enter_context(tc.tile_pool(name="p1_w", bufs=1))
            p1_x = ctx1.enter_context(tc.tile_pool(name="p1_x", bufs=1))
            p1_cs = ctx1.enter_context(tc.tile_pool(name="p1_cs", bufs=1))
            p1_st = ctx1.enter_context(tc.tile_pool(name="p1_st", bufs=1))
            p1_ps = ctx1.enter_context(tc.tile_pool(name="p1_ps", bufs=1,
                                                    space="PSUM"))

            def load_x(nt):
                tiles = []
                for tag, srcd in (("xh", xTh_d), ("xl", xTl_d)):
                    xt = p1_x.tile([128, NPAIR * 2 * NT], F8, name=f"x{tag}",
                                   tag=tag, bufs=2)
                    nc.sync.dma_start(xt[:], srcd[nt])
                    tiles.append(xt)
                return tiles

            # startup DMA order: xh0, wh0, xl0, wl0, then the remaining
            # chunks hi/lo interleaved -- matches the chain emission order so
            # PE starts as soon as the first hi chunk lands
            def _wtile(tag, q):
                return p1_w.tile([128, (NPAIR // NCH) * 2 * NROW], F8,
                                 name=f"{tag}{q}", tag=f"{tag}{q}", bufs=1)

            xh0 = p1_x.tile([128, NPAIR * 2 * NT], F8, name="xxh", tag="xh",
                            bufs=2)
            HX = NPAIR * NT           # half the x tile
            HW_ = (NPAIR // NCH) * NROW
            # first tiles stream in halves: subtile deps let the first
            # chains start one half-DMA earlier
            nc.sync.dma_start(xh0[:, 0:HX], xTh_d[0][:, 0:HX])
            wh_sb = [_wtile("wh", q) for q in range(NCH)]
            wl_sb = [_wtile("wl", q) for q in range(NCH)]
            nc.sync.dma_start(wh_sb[0][:, 0:HW_], wqh_d[0][:, 0:HW_])
            nc.sync.dma_start(xh0[:, HX:2 * HX], xTh_d[0][:, HX:2 * HX])
            nc.sync.dma_start(wh_sb[0][:, HW_:2 * HW_],
                              wqh_d[0][:, HW_:2 * HW_])
            xl0 = p1_x.tile([128, NPAIR * 2 * NT], F8, name="xxl", tag="xl",
                            bufs=2)
            nc.sync.dma_start(xl0[:], xTl_d[0])
            nc.sync.dma_start(wl_sb[0][:], wql_d[0])
            x0 = (xh0, xl0)
            for q in range(1, NCH):
                nc.sync.dma_start(wh_sb[q][:], wqh_d[q])
                nc.sync.dma_start(wl_sb[q][:], wql_d[q])

            PC = NPAIR // NCH     # pairs per weight chunk

            def wv(ws, i, m):     # lhsT [128, 2, 128] for pair i, m-tile m
                return ws[i // PC][:].rearrange(
                    "p (i two c) -> p i two c", two=2, c=NROW)[
                    :, i % PC, :, 128 * m:128 * (m + 1)]

            def xv(xs, i):        # rhs [128, 2, NT]
                return xs[:].rearrange("p (i two c) -> p i two c",
                                       i=NPAIR, two=2)[:, i]

            for nt in range(NNT):
                c0 = NT * nt
                xh_sb, xl_sb = x0 if nt == 0 else load_x(nt)

                csd = p1_cs.tile([128, NT], F32, name="csd", tag="csd", bufs=2)
                nc.sync.dma_start(csd[:], cosP_d[:, c0:c0 + NT])
                snd = p1_cs.tile([128, NT], F32, name="snd", tag="snd", bufs=2)
                nc.sync.dma_start(snd[:], sinP_d[:, c0:c0 + NT])

                stage = p1_st.tile([128, (NM - 1) * NT], BF16, name="stage",
                                   tag="stage", bufs=2)
                # all 3 m-pair chain sets live at once (6 psum banks) so the
                # matmuls can be emitted chunk-major: each weight chunk is
                # fully consumed as it arrives, which keeps PE fed during the
                # startup weight stream
                pas, pls = [], []
                for mp in range(NM // 2):
                    pas.append(p1_ps.tile([128, 2 * NT], F32, name="pa",
                                          tag="pa", bufs=3))
                    pls.append(p1_ps.tile([128, 2 * NT], F32, name="pl",
                                          tag="pl", bufs=3))
                for q in range(NCH):
                    i0, i1 = PC * q, PC * q + PC
                    for chain in range(3):
                        for mp in range(NM // 2):
                            for half, m in ((0, 2 * mp), (1, 2 * mp + 1)):
                                cc = NT * half
                                for i in range(i0, i1):
                                    # NB: exactly ONE start=True per psum
                                    # tile -- start marks the WHOLE 2KB bank
                                    # pending-zero, so the other half's first
                                    # write replaces (never re-mark mid-chain)
                                    if chain == 0:
                                        nc.tensor.matmul(
                                            pas[mp][:, cc:cc + NT],
                                            wv(wh_sb, i, m), xv(xh_sb, i),
                                            start=(i == 0 and half == 0),
                                            stop=(i == NPAIR - 1),
                                            perf_mode=PM.DoubleRow,
                                            skip_group_check=True)
                                    elif chain == 1:
                                        nc.tensor.matmul(
                                            pls[mp][:, cc:cc + NT],
                                            wv(wh_sb, i, m), xv(xl_sb, i),
                                            start=(i == 0 and half == 0),
                                            stop=False,
                                            perf_mode=PM.DoubleRow,
                                            skip_group_check=True)
                                    else:
                                        nc.tensor.matmul(
                                            pls[mp][:, cc:cc + NT],
                                            wv(wl_sb, i, m), xv(xh_sb, i),
                                            start=False,
                                            stop=(i == NPAIR - 1),
                                            perf_mode=PM.DoubleRow,
                                            skip_group_check=True)
                for mp in range(NM // 2):
                    m0 = 2 * mp
                    pa, pl = pas[mp], pls[mp]
                    # combine hi + lo/32 -> 64*qkv_true; scale on ACT
                    # (a DVE op may read at most one PSUM operand)
                    tl = p1_st.tile([128, 2 * NT], F32, name="tl",
                                    tag="tl", bufs=2)
                    nc.scalar.activation(tl[:], pl[:], AT.Copy,
                                         bias=0.0, scale=1.0 / SL)
                    a_sb = p1_st.tile([128, 2 * NT], F32R, name="a_sb",
                                      tag="a_sb", bufs=2)
                    nc.vector.tensor_tensor(a_sb[:], tl[:], pa[:], op=OP.add)
                    rw = 2 * NT if mp < 2 else NT   # rope width (m=5 is v)
                    pb = p1_ps.tile([128, 2 * NT], F32, name="pb", tag="pb",
                                    bufs=1)
                    nc.tensor.matmul(pb[:, 0:rw], pswapT_sb[:], a_sb[:, 0:rw],
                                     start=True, stop=True)
                    tcos = p1_st.tile([128, 2 * NT], F32, name="tcos",
                                      tag="tcos", bufs=2)
                    tsin = p1_st.tile([128, 2 * NT], F32, name="tsin",
                                      tag="tsin", bufs=2)
                    if rw == 2 * NT:
                        # cos/sin loaded once; 0-stride broadcast over halves
                        csb = csd[:].unsqueeze(1).to_broadcast([128, 2, NT])
                        snb = snd[:].unsqueeze(1).to_broadcast([128, 2, NT])
                        nc.vector.tensor_tensor(
                            tcos[:].rearrange("p (two c) -> p two c", two=2),
                            a_sb[:].bitcast(F32).rearrange(
                                "p (two c) -> p two c", two=2),
                            csb, op=OP.mult)
                        nc.vector.tensor_tensor(
                            tsin[:].rearrange("p (two c) -> p two c", two=2),
                            pb[:].rearrange("p (two c) -> p two c", two=2),
                            snb, op=OP.mult)
                    else:
                        nc.vector.tensor_tensor(tcos[:, 0:rw],
                                                a_sb[:, 0:rw].bitcast(F32),
                                                csd[:], op=OP.mult)
                        nc.vector.tensor_tensor(tsin[:, 0:rw], pb[:, 0:rw],
                                                snd[:], op=OP.mult)
                    nc.vector.tensor_tensor(
                        stage[:, NT * m0:NT * m0 + rw],
                        tcos[:, 0:rw], tsin[:, 0:rw], op=OP.add)
                    if mp == 2:
                        # vT (64x scale, folded into y) -> v natural via PE;
                        # both transposes share one psum bank (disjoint cols)
                        vpack = p1_st.tile([128, NT], BF16, name="vpack",
                                           tag="vpack", bufs=2)
                        pt = p1_ps.tile([128, 2 * 128], F32, name="pt",
                                        tag="pt", bufs=1)
                        for j in range(NT // 128):
                            nc.tensor.transpose(
                                pt[:, 128 * j:128 * (j + 1)],
                                a_sb[:, NT + 128 * j:NT + 128 * (j + 1)]
                                .bitcast(F32),
                                ident_sb[:])
                            nc.vector.tensor_copy(
                                vpack[:, 128 * j:128 * (j + 1)],
                                pt[:, 128 * j:128 * (j + 1)])
                        nc.gpsimd.dma_start(
                            v_t[c0 // S][c0 % S:c0 % S + NT, :]
                            .rearrange("(j p) d -> p j d", p=128),
                            vpack[:].rearrange("p (j d) -> p j d", d=128))
                # one packed DMA for the 5 roped m-tiles
                nc.gpsimd.dma_start(
                    qkvT_t[c0 // S][0:(NM - 1) * 128, c0 % S:c0 % S + NT]
                    .rearrange("(m p) c -> p m c", p=128),
                    stage[:].rearrange("p (m c) -> p m c", c=NT))

        # ================= phase 4 pools opened early for prefetch
        p4_w = glob.enter_context(tc.tile_pool(name="p4_w", bufs=1))
        p4_y = glob.enter_context(tc.tile_pool(name="p4_y", bufs=1))

        wo_sb = {}

        def load_wo(do):
            # ACT-queue dispatch: paced behind the phase-2 exps so the 2MB
            # wo streams never starve phase-2's latency-critical loads
            tiles = {}
            for tag, srcd in (("h", woh_d), ("l", wol_d)):
                wt = p4_w.tile([128, NPAIR * 2 * WO_NT], F8,
                               name=f"wo{tag}{do}", tag=f"wo{tag}", bufs=2)
                nc.gpsimd.dma_start(wt[:], srcd[do])
                tiles[tag] = wt
            wo_sb[do] = tiles

        y_big = {}

        def load_y(buf):
            for tag, r0 in (("h", 0), ("l", 256)):
                yt = p4_y.tile([128, 8 * 2 * TPC], F8, name=f"y{tag}{buf}",
                               tag=f"y{tag}{buf}", bufs=1)
                for c in range(8):
                    nc.sync.dma_start(
                        yt[:, 2 * TPC * c:2 * TPC * (c + 1)]
                        .rearrange("p (i t) -> p i t", i=2),
                        a2a_out[buf][512 * c + r0:512 * c + r0 + 256, :]
                        .rearrange("(i p) t -> p i t", p=128))
                y_big[(tag, buf)] = yt

        # ================= phase 2: attention (bf16), head-pair outer
        with ExitStack() as ctx2:
            p2_ps = ctx2.enter_context(tc.tile_pool(name="p2_ps", bufs=1,
                                                    space="PSUM"))
            if mode == "generic":
                p2_bias = ctx2.enter_context(tc.tile_pool(name="p2_bias", bufs=4))

            # kT / v for both batches load up-front (cached across
            # head-pairs); qT loads run one head ahead of use.  All phase-2
            # loads ride the ACT HWDGE queue so they dispatch while phase 1
            # still owns the SP queue with its scratch writes.
            # tag-order pins psc to the psum banks phase 1 frees first
            p2_ps.tile([128, 2 * QT_W], F32, name="ord0", tag="psc", bufs=2)
            p2_ps.tile([128, QT_W], F32, name="ord1", tag="py", bufs=2)
            p2_ps.tile([128, 512], F32, name="ord2", tag="sums", bufs=2)

            kT_sb, v_sb = {}, {}

            def load_kv(b):
                eng = nc.scalar
                kT = p2_qk.tile([128, S], BF16, name=f"kT{b}",
                                tag=f"kT{b}", bufs=1)
                eng.dma_start(
                    kT[:], qkvT_t[b][QH * 128:(QH + 1) * 128, :])
                kT_sb[b] = kT
                vts = []
                for i in range(N_KB // 4):
                    vt = p2_v.tile([128, 4 * HD], BF16, name=f"v{b}_{i}",
                                   tag=f"v{b}_{i}", bufs=1)
                    eng.dma_start(
                        vt[:].rearrange("p (j d) -> p j d", d=HD),
                        v_t[b][512 * i:512 * (i + 1), :]
                        .rearrange("(j p) d -> p j d", p=128))
                    vts.append(vt)
                v_sb[b] = vts

            load_kv(0)

            QT_ORD = (3, 2, 1, 0)
            head_order = [(hp, b, 2 * hp + hh)
                          for hp in range(2) for b in range(B)
                          for hh in range(2)]
            qT_sb = {}

            def load_qT(k):
                hp_, b_, h_ = head_order[k]
                qT = p2_qk.tile([128, S], BF16, name="qT", tag="qT", bufs=3)
                nc.scalar.dma_start(
                    qT[:], qkvT_t[b_][128 * h_:128 * (h_ + 1), :])
                qT_sb[k] = qT

            load_qT(0)
            # deferred per-qt normalize tails: emitted one group into the
            # NEXT q-tile so the recip->transpose->broadcast chain never
            # parks at the head of PE's in-order queue
            tails = []

            def flush_tail():
                while tails:
                    tails.pop(0)()

            def make_tail(py, sums, hp, b, h, qt):
                def tail():
                    # normalize: y64 = py * (1/sums); py carries the 64x v
                    # scale, ones=4 -> y16 = 16*y -> fp8 hi/lo split
                    sr = p2_ms.tile([128, 4], F32R, name="sr",
                                    tag="sr", bufs=2)
                    with nc.allow_low_precision(reason="f32r recip"):
                        nc.vector.reciprocal(sr[:], sums[:, 508:512])
                    # per-column PE transposes land the recips as ONE
                    # partition-0 [1,512] psum row (the gpsimd broadcast
                    # requires its source at partition 0)
                    for qb in range(4):
                        nc.tensor.transpose(
                            sums[0:1, 128 * qb:128 * (qb + 1)].bitcast(F32R),
                            sr[:, qb:qb + 1],
                            ident_sb[:].bitcast(F32R))
                    rT = p2_ms.tile([1, QT_W], F32R, name="rT",
                                    tag="rT", bufs=2)
                    nc.vector.tensor_copy(rT[0:1, :],
                                          sums[0:1, 0:512].bitcast(F32R))
                    rep_sb = p2_ms.tile([128, QT_W], F32R,
                                        name="rep_sb", tag="rep", bufs=2)
                    nc.gpsimd.partition_broadcast(rep_sb[:], rT[0:1, :],
                                                  channels=128)
                    yT_sb = p2_ms.tile([128, QT_W], F32, name="yT_sb",
                                       tag="yT", bufs=2)
                    nc.vector.tensor_tensor(yT_sb[:], py[:], rep_sb[:],
                                            op=OP.mult)
                    yh_sb = p2_ms.tile([128, QT_W], F8, name="yh_sb",
                                       tag="yh", bufs=2)
                    nc.vector.tensor_copy(yh_sb[:], yT_sb[:])
                    yr_sb = p2_ms.tile([128, QT_W], F32, name="yr_sb",
                                       tag="yr", bufs=2)
                    nc.vector.tensor_tensor(yr_sb[:], yT_sb[:],
                                            yh_sb[:], op=OP.subtract)
                    yl_sb = p2_ms.tile([128, QT_W], F8, name="yl_sb",
                                       tag="yl", bufs=2)
                    nc.vector.tensor_scalar_mul(yl_sb[:], yr_sb[:], SL)
                    # A2A chunk j = 4b + qt: hi rows 512j+128*(h%2),
                    # lo rows 512j+256+128*(h%2)
                    j = 4 * b + qt
                    r0 = 512 * j + 128 * (h % 2)
                    nc.sync.dma_start(
                        a2a_in[hp][r0:r0 + 128, :], yh_sb[:])
                    nc.sync.dma_start(
                        a2a_in[hp][r0 + 256:r0 + 384, :], yl_sb[:])
                return tail

            def emit_first_block(ho_t, qt_t):
                """Scores [+mask] + exp for the FIRST k-block pair of
                (ho_t, qt_t); emitted at the END of the previous q-tile so
                ACT stays fed across qt/head boundaries."""
                hp_t, b_t, h_t = head_order[ho_t]
                kT_t, qT_t = kT_sb[b_t], qT_sb[ho_t]
                n_full_t = 4 * qt_t if causal else N_KB
                psc = p2_ps.tile([128, 2 * QT_W], F32, name="psc",
                                 tag="psc", bufs=2)
                at = p2_at.tile([128, 2 * QT_W], BF16, name="at",
                                tag="at", bufs=3)
                if causal and n_full_t == 0:
                    for o, dj in ((0, 0), (QT_W, 1)):
                        w = QT_W - 128 * dj
                        nc.tensor.matmul(
                            psc[:, o:o + w],
                            kT_t[:, 128 * dj:128 * (dj + 1)],
                            qT_t[:, 128 * dj:QT_W],
                            start=True, stop=False,
                            skip_group_check=True)
                        nc.tensor.matmul(
                            psc[:, o:o + 128], trinegT_sb[:], identb_sb[:],
                            start=False, stop=True,
                            skip_group_check=True)
                    nc.scalar.activation(at[:, 0:2 * QT_W - 128],
                                         psc[:, 0:2 * QT_W - 128], AT.Exp,
                                         bias=-float(c_sub),
                                         scale=float(SCALE))
                    return ("diag0", at)
                for kk in (0, 1):
                    nc.tensor.matmul(
                        psc[:, QT_W * kk:QT_W * (kk + 1)],
                        kT_t[:, 128 * kk:128 * (kk + 1)],
                        qT_t[:, QT_W * qt_t:QT_W * (qt_t + 1)],
                        start=True, stop=True, skip_group_check=True)
                    if mode == "generic":
                        bt = p2_bias.tile([128, QT_W], F32, name="bt")
                        nc.sync.dma_start(
                            bt[:], biasT_d[128 * kk:128 * (kk + 1),
                                           QT_W * qt_t:QT_W * (qt_t + 1)])
                        nc.vector.tensor_tensor(
                            psc[:, QT_W * kk:QT_W * (kk + 1)],
                            psc[:, QT_W * kk:QT_W * (kk + 1)],
                            bt[:], op=OP.add)
                nc.scalar.activation(at[:], psc[:], AT.Exp,
                                     bias=-float(c_sub), scale=float(SCALE))
                return ("full0", at)

            def emit_group_scores(ho_t, qt_t, g):
                hp_t, b_t, h_t = head_order[ho_t]
                kT_t, qT_t = kT_sb[b_t], qT_sb[ho_t]
                psc = p2_ps.tile([128, 2 * QT_W], F32, name="psc",
                                 tag="psc", bufs=2)
                at = p2_at.tile([128, 2 * QT_W], BF16, name="at",
                                tag="at", bufs=3)
                for kk, kb in ((0, 2 * g), (1, 2 * g + 1)):
                    nc.tensor.matmul(
                        psc[:, QT_W * kk:QT_W * (kk + 1)],
                        kT_t[:, 128 * kb:128 * (kb + 1)],
                        qT_t[:, QT_W * qt_t:QT_W * (qt_t + 1)],
                        start=True, stop=True, skip_group_check=True)
                    if mode == "generic":
                        bt = p2_bias.tile([128, QT_W], F32, name="bt")
                        nc.sync.dma_start(
                            bt[:], biasT_d[128 * kb:128 * (kb + 1),
                                           QT_W * qt_t:QT_W * (qt_t + 1)])
                        nc.vector.tensor_tensor(
                            psc[:, QT_W * kk:QT_W * (kk + 1)],
                            psc[:, QT_W * kk:QT_W * (kk + 1)],
                            bt[:], op=OP.add)
                nc.scalar.activation(at[:], psc[:], AT.Exp,
                                     bias=-float(c_sub), scale=float(SCALE))
                return at

            pre = None
            for ho, (hp, b, h) in enumerate(head_order):
                if ho + 1 < len(head_order):
                    load_qT(ho + 1)
                if ho == 1:
                    load_kv(1)
                kT = kT_sb[b]
                qT = qT_sb[ho]

                def vvw(kb, b=b):
                    return v_sb[b][kb // 4][:, HD * (kb % 4):
                                            HD * (kb % 4) + HD]

                # q-tiles run 3,2,1,0: the LONG tile leads each head so the
                # previous head's deferred normalize tail (py-slot WAR, two
                # tiles deep) is absorbed instead of stalling a short qt0
                for qi, qt in enumerate(QT_ORD):
                    kb_max = 4 * qt + 4 if causal else N_KB
                    n_full = 4 * qt if causal else kb_max
                    py = p2_ps.tile([128, QT_W], F32, name="py",
                                    tag="py", bufs=2)
                    # cols 508:512 = denominator chains; the partition-0
                    # row cols 0:512 hosts the transposed recips
                    sums = p2_ps.tile([128, 512], F32, name="sums",
                                      tag="sums", bufs=2)
                    if pre is None:
                        pre = [emit_first_block(ho, qt)]
                    kind, at0 = pre[0]
                    pre_g1 = pre[1][1] if len(pre) > 1 else None
                    pre = None
                    flush_tail()
                    # ---- AV + denominator chains for the pre-built block
                    if kind == "diag0":
                        for o, dj in ((0, 0), (QT_W, 1)):
                            kb = dj
                            nc.tensor.matmul(
                                py[:, 128 * dj:QT_W], vvw(kb),
                                at0[:, o:o + QT_W - 128 * dj],
                                start=(dj == 0), stop=(False),
                                skip_group_check=True)
                            for qb in range(dj, 4):
                                nc.tensor.matmul(
                                    sums[:, 508 + qb:509 + qb],
                                    at0[:, o + 128 * (qb - dj):
                                        o + 128 * (qb - dj) + 128],
                                    ones_col_sb[:],
                                    start=(dj == 0 and qb == 0),
                                    stop=(qb == dj),
                                    skip_group_check=True)
                    else:
                        for kk, kb in ((0, 0), (1, 1)):
                            nc.tensor.matmul(
                                py[:], vvw(kb),
                                at0[:, QT_W * kk:QT_W * (kk + 1)],
                                start=(kb == 0),
                                stop=(not causal and kb == n_full - 1),
                                skip_group_check=True)
                            for qb in range(4):
                                nc.tensor.matmul(
                                    sums[:, 508 + qb:509 + qb],
                                    at0[:, QT_W * kk + 128 * qb:
                                        QT_W * kk + 128 * qb + 128],
                                    ones_col_sb[:],
                                    start=(kb == 0 and qb == 0),
                                    stop=(not causal and kb == n_full - 1),
                                    skip_group_check=True)
                    if pre_g1 is not None:
                        for kk, kb in ((0, 2), (1, 3)):
                            nc.tensor.matmul(
                                py[:], vvw(kb),
                                pre_g1[:, QT_W * kk:QT_W * (kk + 1)],
                                start=False,
                                stop=(not causal and kb == n_full - 1),
                                skip_group_check=True)
                            for qb in range(4):
                                nc.tensor.matmul(
                                    sums[:, 508 + qb:509 + qb],
                                    pre_g1[:, QT_W * kk + 128 * qb:
                                           QT_W * kk + 128 * qb + 128],
                                    ones_col_sb[:],
                                    start=False,
                                    stop=(not causal and kb == n_full - 1),
                                    skip_group_check=True)
                    # ---- remaining full k-block pairs
                    g_done = (2 if pre_g1 is not None else
                              (1 if kind == "full0" else 0))
                    for g in range(g_done, n_full // 2):
                        kbs = (2 * g, 2 * g + 1)
                        psc = p2_ps.tile([128, 2 * QT_W], F32,
                                         name="psc", tag="psc", bufs=2)
                        for kk, kb in enumerate(kbs):
                            nc.tensor.matmul(
                                psc[:, QT_W * kk:QT_W * (kk + 1)],
                                kT[:, 128 * kb:128 * (kb + 1)],
                                qT[:, QT_W * qt:QT_W * (qt + 1)],
                                start=True, stop=True,
                                skip_group_check=True)
                            if mode == "generic":
                                bt = p2_bias.tile([128, QT_W], F32,
                                                  name="bt")
                                nc.sync.dma_start(
                                    bt[:],
                                    biasT_d[128 * kb:128 * (kb + 1),
                                            QT_W * qt:QT_W * (qt + 1)])
                                nc.vector.tensor_tensor(
                                    psc[:, QT_W * kk:QT_W * (kk + 1)],
                                    psc[:, QT_W * kk:QT_W * (kk + 1)],
                                    bt[:], op=OP.add)
                        at = p2_at.tile([128, 2 * QT_W], BF16,
                                        name="at", tag="at", bufs=3)
                        nc.scalar.activation(at[:], psc[:], AT.Exp,
                                             bias=-float(c_sub),
                                             scale=float(SCALE))
                        for kk, kb in enumerate(kbs):
                            nc.tensor.matmul(
                                py[:], vvw(kb),
                                at[:, QT_W * kk:QT_W * (kk + 1)],
                                start=False,
                                stop=(not causal and kb == n_full - 1),
                                skip_group_check=True)
                            for qb in range(4):
                                nc.tensor.matmul(
                                    sums[:, 508 + qb:509 + qb],
                                    at[:, QT_W * kk + 128 * qb:
                                        QT_W * kk + 128 * qb + 128],
                                    ones_col_sb[:],
                                    start=False,
                                    stop=(not causal and kb == n_full - 1),
                                    skip_group_check=True)
                    # ---- remaining causal diag pairs (exact widths)
                    dps = ((1,) if kind == "diag0" else (0, 1)) if causal \
                        else ()
                    for dp in dps:
                        djs = (2 * dp, 2 * dp + 1)
                        ws = [QT_W - 128 * dj for dj in djs]
                        offs = [0, ws[0]]
                        psc = p2_ps.tile([128, 2 * QT_W], F32,
                                         name="psc", tag="psc", bufs=2)
                        for o, dj, w in zip(offs, djs, ws):
                            kb = n_full + dj
                            nc.tensor.matmul(
                                psc[:, o:o + w],
                                kT[:, 128 * kb:128 * (kb + 1)],
                                qT[:, QT_W * qt + 128 * dj:QT_W * (qt + 1)],
                                start=True, stop=False,
                                skip_group_check=True)
                            nc.tensor.matmul(
                                psc[:, o:o + 128], trinegT_sb[:],
                                identb_sb[:],
                                start=False, stop=True,
                                skip_group_check=True)
                        at = p2_at.tile([128, 2 * QT_W], BF16,
                                        name="at", tag="at", bufs=3)
                        nc.scalar.activation(at[:, 0:ws[0] + ws[1]],
                                             psc[:, 0:ws[0] + ws[1]], AT.Exp,
                                             bias=-float(c_sub),
                                             scale=float(SCALE))
                        for o, dj, w in zip(offs, djs, ws):
                            kb = n_full + dj
                            cc0 = 128 * dj
                            nc.tensor.matmul(
                                py[:, cc0:QT_W], vvw(kb), at[:, o:o + w],
                                start=False, stop=(dj == 3),
                                skip_group_check=True)
                            for qb in range(dj, 4):
                                nc.tensor.matmul(
                                    sums[:, 508 + qb:509 + qb],
                                    at[:, o + 128 * (qb - dj):
                                        o + 128 * (qb - dj) + 128],
                                    ones_col_sb[:],
                                    start=False, stop=(qb == dj),
                                    skip_group_check=True)
                    # ---- pre-emit the NEXT q-tile's first block
                    if not (ho == len(head_order) - 1
                            and qi == N_QT - 1):
                        nxt = ((ho, QT_ORD[qi + 1]) if qi + 1 < N_QT
                               else (ho + 1, QT_ORD[0]))
                        pre = [emit_first_block(*nxt)]
                        nf_nxt = 4 * nxt[1] if causal else N_KB
                        if pre[0][0] == "full0" and nf_nxt // 2 >= 2:
                            pre.append(
                                ("full1", emit_group_scores(*nxt, 1)))
                    tails.append(make_tail(py, sums, hp, b, h, qt))
                # ---- head-pair boundary: A2A (overlaps later compute)
                if ho in (3, 7):
                    flush_tail()
                    if sim:
                        for j in range(CORES):
                            nc.sync.dma_start(
                                a2a_out[hp][512 * j:512 * (j + 1), :],
                                a2a_in[hp][512 * j:512 * (j + 1), :])
                    else:
                        nc.gpsimd.collective_compute(
                            "AllToAll", mybir.AluOpType.bypass,
                            replica_groups=[list(range(CORES))],
                            ins=[a2a_in[hp][:]], outs=[a2a_out[hp][:]],
                        )
                    if hp == 0:
                        load_y(0)  # first A2A's payload + first wo blocks
                        load_wo(0)
                        load_wo(1)
            load_y(1)

        # ===== phase 4: out = y @ wo.T, fp8 3-term over kb pairs
        # pair j = (buf j%2, chunk j//2) = y rows [512(j//2)+..] of a2a buf;
        # even pairs (A2A #1) first, per-psum chains split even/odd so the
        # even halves of all 4 token blocks run while A2A #2 lands.
        evens = [j for j in range(NPAIR) if j % 2 == 0]
        odds = [j for j in range(NPAIR) if j % 2 == 1]

        def yv(tag, j, tb):   # lhsT [128, 2, 128] for pair j
            return y_big[(tag, j % 2)][:].rearrange(
                "p (c i t) -> p c i t", c=8, i=2)[
                :, j // 2, :, 128 * tb:128 * (tb + 1)]

        with ExitStack() as ctx4:
            p4_st = ctx4.enter_context(tc.tile_pool(name="p4_st", bufs=1))
            p4_ps = ctx4.enter_context(tc.tile_pool(name="p4_ps", bufs=1,
                                                    space="PSUM"))

            for do in range(D // WO_NT):
                if do + 2 < D // WO_NT:
                    load_wo(do + 2)
                wo_t = wo_sb[do]

                def wov(tag, i):  # rhs [128, 2, WO_NT]
                    return wo_t[tag][:].rearrange(
                        "p (i two c) -> p i two c", i=NPAIR, two=2)[:, i]

                pos, pols = [], []
                for tb in range(TPC // 128):
                    po = p4_ps.tile([128, WO_NT], F32, name="po", tag="po",
                                    bufs=4)
                    pol = p4_ps.tile([128, WO_NT], F32, name="pol", tag="pol",
                                     bufs=4)
                    for n, j in enumerate(evens):
                        nc.tensor.matmul(po[:], yv("h", j, tb), wov("h", j),
                                         start=(n == 0), stop=False,
                                         perf_mode=PM.DoubleRow,
                                         skip_group_check=True)
                    for n, j in enumerate(evens):
                        nc.tensor.matmul(pol[:], yv("l", j, tb), wov("h", j),
                                         start=(n == 0), stop=False,
                                         perf_mode=PM.DoubleRow,
                                         skip_group_check=True)
                    for j in evens:
                        nc.tensor.matmul(pol[:], yv("h", j, tb), wov("l", j),
                                         start=False, stop=False,
                                         perf_mode=PM.DoubleRow,
                                         skip_group_check=True)
                    pos.append(po)
                    pols.append(pol)
                for tb in range(TPC // 128):
                    po, pol = pos[tb], pols[tb]
                    for n, j in enumerate(odds):
                        nc.tensor.matmul(po[:], yv("h", j, tb), wov("h", j),
                                         start=False, stop=(n == len(odds) - 1),
                                         perf_mode=PM.DoubleRow,
                                         skip_group_check=True)
                    for j in odds:
                        nc.tensor.matmul(pol[:], yv("l", j, tb), wov("h", j),
                                         start=False, stop=False,
                                         perf_mode=PM.DoubleRow,
                                         skip_group_check=True)
                    for n, j in enumerate(odds):
                        nc.tensor.matmul(pol[:], yv("h", j, tb), wov("l", j),
                                         start=False, stop=(n == len(odds) - 1),
                                         perf_mode=PM.DoubleRow,
                                         skip_group_check=True)
                    # out = (hi + lo/32) / (16*64); y carries 16x, wo 64x
                    tl4 = p4_st.tile([128, WO_NT], F32, name="tl4",
                                     tag="tl4", bufs=2)
                    nc.vector.tensor_scalar_mul(tl4[:], pol[:], 1.0 / SL)
                    ts = p4_st.tile([128, WO_NT], F32, name="ts", tag="ts",
                                    bufs=2)
                    nc.vector.tensor_tensor(ts[:], tl4[:], po[:], op=OP.add)
                    o_sb = p4_st.tile([128, WO_NT], F32, name="o_sb",
                                      tag="o_sb", bufs=2)
                    nc.scalar.activation(o_sb[:], ts[:], AT.Copy,
                                         bias=0.0, scale=1.0 / (16.0 * SW))
                    # out writes dispatch from SP (idle in phase 4) so the
                    # final write never queues behind ACT's o_sb scale
                    nc.sync.dma_start(
                        out_d[128 * tb:128 * (tb + 1),
                              WO_NT * do:WO_NT * (do + 1)],
                        o_sb[:])

    nc.compile()
    return nc


def _prepare(x, freqs_cis, mask, wqkv_w, wo_w):
    """Host-side prep: mode detection, stability constant, fp8 input maps."""
    E4 = ml_dtypes.float8_e4m3
    x = np.asarray(x, dtype=np.float32)
    freqs_cis = np.asarray(freqs_cis, dtype=np.float32)
    mask = np.asarray(mask)
    wqkv_w = np.asarray(wqkv_w, dtype=np.float32)
    wo_w = np.asarray(wo_w, dtype=np.float32)

    m2 = mask.reshape(mask.shape[-2], mask.shape[-1])
    if np.array_equal(m2, np.tril(np.ones((S, S), dtype=bool))):
        mode = "causal"
    elif m2.all():
        mode = "full"
    else:
        mode = "generic"

    def split(a, s_hi):
        hi = (a * s_hi).astype(E4)
        lo = ((a * s_hi - hi.astype(np.float32)) * SL).astype(E4)
        return hi, lo

    x2 = x.reshape(TOK, D)
    xT = np.ascontiguousarray(x2.T)
    xTh, xTl = split(xT, 1.0)

    def blk_x(a):   # [D, TOK] -> [NNT, 128, NPAIR*2*NT] contiguous loads
        a = a.reshape(NPAIR, 2, 128, NNT, NT)
        return np.ascontiguousarray(
            a.transpose(3, 2, 0, 1, 4).reshape(NNT, 128, NPAIR * 2 * NT))

    xTh, xTl = blk_x(xTh), blk_x(xTl)

    woT = np.ascontiguousarray(wo_w.T)
    woh, wol = split(woT, SW)

    def blk_wo(a):  # [D, D] -> [8, 128, NPAIR*2*WO_NT]
        a = a.reshape(NPAIR, 2, 128, D // WO_NT, WO_NT)
        return np.ascontiguousarray(
            a.transpose(3, 2, 0, 1, 4).reshape(D // WO_NT, 128,
                                               NPAIR * 2 * WO_NT))

    woh, wol = blk_wo(woh), blk_wo(wol)

    cos = freqs_cis[:, :, 0].T          # [64, S]
    sin = freqs_cis[:, :, 1].T
    cosP = np.repeat(cos, 2, axis=0)    # [128, S]
    sinP = np.repeat(sin, 2, axis=0)
    # 1/64 dequant of the qkv projection rides on the rope coefficients
    cosP = np.ascontiguousarray(np.tile(cosP, (1, B))) / np.float32(SW)
    sinP = np.ascontiguousarray(np.tile(sinP, (1, B))) / np.float32(SW)

    # softmax stability probe: rope'd scores for head 0, batch 0, 128 q rows
    wq0 = wqkv_w[:HD]                   # [128, D]
    wk0 = wqkv_w[NH * HD:NH * HD + HD]  # [128, D]
    qs = x2[:128] @ wq0.T               # [128, 128]
    ks = x2[:S] @ wk0.T                 # [S, 128]

    def rope_np(t, fc):
        ts = t.reshape(t.shape[0], HD // 2, 2)
        c, s_ = fc[:t.shape[0], :, 0], fc[:t.shape[0], :, 1]
        out = np.empty_like(ts)
        out[:, :, 0] = ts[:, :, 0] * c - ts[:, :, 1] * s_
        out[:, :, 1] = ts[:, :, 1] * c + ts[:, :, 0] * s_
        return out.reshape(t.shape)

    qs = rope_np(qs, freqs_cis)
    ks = rope_np(ks, freqs_cis)
    smax = float(np.max(np.abs(qs @ ks.T)) * SCALE)
    c_sub = 0.0 if smax < 25.0 else smax + 5.0

    def blk_w(a):   # [D, NROW] -> [NCH, 128, (NPAIR//NCH)*2*NROW]
        a = a.reshape(NCH, NPAIR // NCH, 2, 128, NROW)
        return np.ascontiguousarray(
            a.transpose(0, 3, 1, 2, 4).reshape(NCH, 128,
                                               (NPAIR // NCH) * 2 * NROW))

    in_maps = []
    for c in range(CORES):
        wq_c = wqkv_w[QH * HD * c:QH * HD * (c + 1)]
        wk_c = wqkv_w[NH * HD + HD * c:NH * HD + HD * (c + 1)]
        wv_c = wqkv_w[(NH + NL) * HD + HD * c:(NH + NL) * HD + HD * (c + 1)]
        wqkvT_c = np.ascontiguousarray(np.vstack([wq_c, wk_c, wv_c]).T)
        wqh, wql = split(wqkvT_c, SW)
        m = {"xTh": xTh, "xTl": xTl, "wqh": blk_w(wqh), "wql": blk_w(wql),
             "woh": woh, "wol": wol, "cosP": cosP, "sinP": sinP}
        if mode == "generic":
            m["biasT"] = np.ascontiguousarray(
                np.where(m2.T, np.float32(0), np.float32(-1e30)))
        in_maps.append(m)
    return mode, c_sub, in_maps


def _get_nc(mode, c_sub):
    key = (mode, round(float(c_sub), 3))
    if key not in _CACHE:
        _CACHE[key] = _build_nc(mode, c_sub)
    return _CACHE[key]


def kernel(x, freqs_cis, mask, wqkv_w, wo_w):
    from concourse import bass_utils
    mode, c_sub, in_maps = _prepare(x, freqs_cis, mask, wqkv_w, wo_w)
    nc = _get_nc(mode, c_sub)
    res = bass_utils.run_bass_kernel_spmd(nc, in_maps, core_ids=list(range(CORES)))
    out = np.concatenate([res.results[c]["out"] for c in range(CORES)], axis=0)
    return out.reshape(B, S, D)
